# revision 1
# baseline (speedup 1.0000x reference)
"""Trainium2 Bass kernel for LogicGatedSNN.

Math:
  w = ternarize(synapse_states)            # {-1,0,1}, exact in bf16
  current = spike_input @ w.T              # bf16 matmul, fp32 PSUM accum -> exact
  spikes[b,o] = (current[b,o] - T[o] >= 0)
where T[o] folds threshold/membrane/refractory:
  non-refractory: T = thr - DECAY*vmem
  refractory:     T = +-1e30 depending on sign of (DECAY*vmem - thr)
The -T[o] bias is added as one extra K=1 fp32 matmul into the same PSUM
accumulation group, so the epilogue is a single tensor_scalar is_ge.

Sharding: 8 cores = 2 (batch) x 4 (out_features). Per core:
  spike shard [2048, 4096], synapse shard [1024, 4096].
On-chip dataflow per core:
  - synapse: fp32 DMA in -> DVE ternarize (is_gt / is_lt / sub) -> bf16
    -> xbar DMA-transpose into W[128, 32, OS] (W[p, m, o] = w[o, 128m+p])
  - spike: SWDGE cast-DMA (fp32->bf16) -> xbar transpose S[128, 32, 128]
  - matmul: psum[128b, 512o] accumulates 32 chunks (K=128 each) + bias mm
  - DVE is_ge vs 0 -> fp32 out tile -> DMA out
"""

import sys

if "/opt/trn_rl_repo" not in sys.path:
    sys.path.insert(0, "/opt/trn_rl_repo")

import numpy as np

B, IN, OUT = 4096, 4096, 4096
GB, GO = 2, 4  # core grid: batch x out_features
DECAY = 0.8
_TENSORS = {}


def build_core_program(nc, tc, bs, os_, in_, instance=0):
    """Emit the per-core program. bs/os_/in_ = per-core shard dims."""
    import concourse.mybir as mybir
    from concourse.bass import ts

    FP32 = mybir.dt.float32
    BF16 = mybir.dt.bfloat16
    Op = mybir.AluOpType

    if instance == 0:
        spike = nc.dram_tensor("spike", [bs, in_], FP32, kind="ExternalInput")
        syn = nc.dram_tensor("syn", [os_, in_], FP32, kind="ExternalInput")
        thr = nc.dram_tensor("thr", [1, os_], FP32, kind="ExternalInput")
        vmem = nc.dram_tensor("vmem", [1, os_], FP32, kind="ExternalInput")
        refrac = nc.dram_tensor("refrac", [1, os_], FP32, kind="ExternalInput")
        # spikes are 0/1 -> exact in bf16; host casts back to fp32
        out = nc.dram_tensor("spikes", [bs, os_], BF16, kind="ExternalOutput")
        _TENSORS.update(
            spike=spike, syn=syn, thr=thr, vmem=vmem, refrac=refrac, out=out
        )
    else:
        spike, syn, thr, vmem, refrac, out = (
            _TENSORS["spike"],
            _TENSORS["syn"],
            _TENSORS["thr"],
            _TENSORS["vmem"],
            _TENSORS["refrac"],
            _TENSORS["out"],
        )

    KC = in_ // 128  # contraction chunks
    NB = bs // 128  # batch tiles
    NT = 512  # matmul free dim per o-tile
    NO = os_ // NT  # o-tiles

    with (
        tc.tile_pool(name="wpool", bufs=1) as wpool,
        tc.tile_pool(name="synpool", bufs=2) as synpool,
        tc.tile_pool(name="ternpool", bufs=2) as ternpool,
        tc.tile_pool(name="sppool", bufs=2) as sppool,
        tc.tile_pool(name="spool", bufs=3) as spool,
        tc.tile_pool(name="outpool", bufs=4) as outpool,
        tc.tile_pool(name="miscpool", bufs=1) as miscpool,
        tc.tile_pool(name="pspool", bufs=4, space="PSUM") as pspool,
    ):
        # ---- threshold vector negT[0, o] = -(effective threshold) ----
        tv = miscpool.tile([1, os_], FP32, tag="tv")
        vv = miscpool.tile([1, os_], FP32, tag="vv")
        rv = miscpool.tile([1, os_], FP32, tag="rv")
        nc.sync.dma_start(tv[:], thr[:, :])
        nc.sync.dma_start(vv[:], vmem[:, :])
        nc.sync.dma_start(rv[:], refrac[:, :])
        c0 = miscpool.tile([1, os_], FP32, tag="c0")
        nc.vector.tensor_scalar(c0[:], vv[:], DECAY, None, Op.mult)
        nc.vector.tensor_tensor(c0[:], c0[:], tv[:], Op.subtract)  # decay*v - thr
        big = miscpool.tile([1, os_], FP32, tag="big")
        nc.vector.tensor_scalar(big[:], c0[:], 0.0, None, Op.is_ge)
        nc.vector.tensor_scalar(big[:], big[:], 2e30, -1e30, Op.mult, Op.add)
        r01 = miscpool.tile([1, os_], FP32, tag="r01")
        nc.vector.tensor_scalar(r01[:], rv[:], 0.0, None, Op.is_gt)
        # negT = c0 + r01 * (big - c0)
        nc.vector.tensor_tensor(big[:], big[:], c0[:], Op.subtract)
        nc.vector.tensor_tensor(big[:], big[:], r01[:], Op.mult)
        negT = miscpool.tile([1, os_], FP32, tag="negT")
        nc.vector.tensor_tensor(negT[:], c0[:], big[:], Op.add)
        ones = miscpool.tile([1, 128], FP32, tag="ones")
        nc.vector.memset(ones[:], 1.0)

        # ---- weights: ternarize + transpose into Ws[ot][p, m, o] = w[o, 128m+p]
        # one tensor per o-tile so the first matmuls only wait on Ws[0]
        Ws = [
            wpool.tile([128, KC, NT], BF16, tag=f"W{ot}", name=f"W{ot}")
            for ot in range(NO)
        ]
        for j in range(os_ // 128):
            ot, jj = divmod(j, NT // 128)
            st = synpool.tile([128, in_], FP32, tag="st", name="st")
            nc.sync.dma_start(st[:], syn[ts(j, 128), :])
            ta = ternpool.tile([128, in_], BF16, tag="ta", name="ta")
            tb = ternpool.tile([128, in_], BF16, tag="tb", name="tb")
            nc.vector.tensor_scalar(ta[:], st[:], 1.0, None, Op.is_gt)
            nc.vector.tensor_scalar(tb[:], st[:], -1.0, None, Op.is_lt)
            nc.vector.tensor_tensor(ta[:], ta[:], tb[:], Op.subtract)
            nc.sync.dma_start(Ws[ot][:, :, ts(jj, 128)], ta[:], transpose=True)

        # ---- main sweep over batch tiles ----
        for bt in range(NB):
            sp = sppool.tile([128, in_], BF16, tag="sp", name="sp")
            nc.gpsimd.dma_start(sp[:], spike[ts(bt, 128), :])  # fp32->bf16 cast
            S = spool.tile([128, KC, 128], BF16, tag="S", name="S")
            nc.sync.dma_start(S[:], sp[:], transpose=True)
            pss = [pspool.tile([128, NT], FP32, tag="ps", name="ps") for _ in range(NO)]
            # m-outer / ot-inner: the stationary S[:, m, :] is reused across
            # o-tiles (PE reorder window pulls the single Ldweights ahead)
            for m in range(KC):
                for ot in range(NO):
                    nc.tensor.matmul(
                        pss[ot][:],
                        S[:, m, :],
                        Ws[ot][:, m, :],
                        start=(m == 0),
                        stop=False,
                    )
            for ot in range(NO):
                nc.tensor.matmul(
                    pss[ot][:], ones[:], negT[:, ts(ot, NT)], start=False, stop=True
                )
                ob = outpool.tile([128, NT], BF16, tag="ob", name="ob")
                nc.vector.tensor_scalar(ob[:], pss[ot][:], 0.0, None, Op.is_ge)
                nc.sync.dma_start(out[ts(bt, 128), ts(ot, NT)], ob[:])
    return out


def make_nc(bs=B // GB, os_=OUT // GO, in_=IN, repeat=1):
    from concourse import bacc
    from concourse.tile import TileContext

    nc = bacc.Bacc(trn_type="TRN2")
    with TileContext(nc) as tc:
        for r in range(repeat):
            build_core_program(nc, tc, bs, os_, in_, instance=r)
    nc.compile()
    return nc


_NC_CACHE = {}


def kernel(
    spike_input,
    synapse_states,
    membrane_potential,
    adaptive_threshold,
    refractory_count,
    _return_results=False,
):
    from concourse.bass_utils import run_bass_kernel_spmd

    spike_input = np.ascontiguousarray(np.asarray(spike_input, dtype=np.float32))
    synapse_states = np.ascontiguousarray(np.asarray(synapse_states, dtype=np.float32))
    membrane_potential = np.asarray(membrane_potential, dtype=np.float32)
    adaptive_threshold = np.asarray(adaptive_threshold, dtype=np.float32)
    refractory_count = np.asarray(refractory_count, dtype=np.float32)

    bs, os_ = B // GB, OUT // GO
    if "nc" not in _NC_CACHE:
        _NC_CACHE["nc"] = make_nc(bs, os_, IN)
    nc = _NC_CACHE["nc"]

    in_maps = []
    for c in range(GB * GO):
        bi, oj = divmod(c, GO)
        in_maps.append(
            {
                "spike": spike_input[bi * bs : (bi + 1) * bs],
                "syn": np.ascontiguousarray(
                    synapse_states[oj * os_ : (oj + 1) * os_]
                ),
                "thr": adaptive_threshold[None, oj * os_ : (oj + 1) * os_],
                "vmem": membrane_potential[None, oj * os_ : (oj + 1) * os_],
                "refrac": refractory_count[None, oj * os_ : (oj + 1) * os_],
            }
        )

    res = run_bass_kernel_spmd(nc, in_maps, core_ids=list(range(GB * GO)))

    full = np.empty((B, OUT), dtype=np.float32)
    for c in range(GB * GO):
        bi, oj = divmod(c, GO)
        full[bi * bs : (bi + 1) * bs, oj * os_ : (oj + 1) * os_] = res.results[c][
            "spikes"
        ].astype(np.float32)
    if _return_results:
        return full, res
    return full



# revision 13
# speedup vs baseline: 1.3833x; 1.3833x over previous
"""Trainium2 Bass kernel for LogicGatedSNN.

Math:
  w = ternarize(synapse_states)            # {-1,0,1}, exact in fp8
  current = spike_input @ w.T              # fp8 matmul, fp32 PSUM accum
  spikes[b,o] = (current[b,o] - T[o] >= 0)
where T[o] folds threshold/membrane/refractory:
  non-refractory: T = thr - DECAY*vmem
  refractory:     T = +-1e30 depending on sign of (DECAY*vmem - thr)

The kernel computes psum[o, b] = -current (the ternarize emits NEGATED
weights so it fits one fused scalar_tensor_tensor op), so the epilogue
is a single per-partition-scalar compare: spike = (psum <= -T[o]).

Sharding: 8 cores = 2 (batch) x 4 (out_features). Per core:
  spikeT shard [4096, 2048] fp8, synT shard [4096, 1024] fp32.
Host prep: spike is exactly {0,1} so the fp8 cast is lossless; both
operands are uploaded pre-transposed (K=IN on rows) so the kernel needs
no on-chip data transposes. Output is produced [o, b] per core and
transposed back on host.

Per-core dataflow:
  - synT: fp32 DMA in (4 m-chunks per DMA) -> 2-op ternarize
    (ta = is_gt(x,1) on Pool; W' = (x is_lt -1) - ta fused on DVE)
    -> negated ternary W'[c][128, 4, 1024] fp8
  - spikeT: fp8 DMA in -> S[c][128, 4, 2048]
  - matmul: plain fp8 (K=128/instr; DoubleRow measured ~30x slower
    Ldweights on real HW, so it is deliberately NOT used): psum
    [128o, 512b] accumulates 32 k-chunks, stationary W'-slice reused
    across 4 moving-S matmuls
  - negT: computed in [8, 128] layout, moved to per-partition [128, 8]
    via the 2-byte hardware transpose unit (bf16 round-trip; exact for
    these magnitudes) -- a rearranged 4B-stride DMA measured ~10ms of
    launch overhead, so it is deliberately NOT used
  - epilogue: is_le(psum, negT[o]) (per-partition scalar) -> fp8 out
"""

import sys

if "/opt/trn_rl_repo" not in sys.path:
    sys.path.insert(0, "/opt/trn_rl_repo")

import numpy as np

B, IN, OUT = 4096, 4096, 4096
GB, GO = 2, 4  # core grid: batch x out_features
DECAY = 0.8
_TENSORS = {}


def build_core_program(nc, tc, bs, os_, in_, instance=0):
    """Emit the per-core program. bs/os_/in_ = per-core shard dims."""
    import concourse.mybir as mybir
    from concourse.bass import ts

    FP32 = mybir.dt.float32
    FP8 = mybir.dt.float8e4
    Op = mybir.AluOpType
    DR = mybir.MatmulPerfMode.DoubleRow

    if instance == 0:
        spt = nc.dram_tensor("spt", [in_, bs], FP8, kind="ExternalInput")
        synt = nc.dram_tensor("synt", [in_, os_], FP32, kind="ExternalInput")
        thr = nc.dram_tensor("thr", [1, os_], FP32, kind="ExternalInput")
        vmem = nc.dram_tensor("vmem", [1, os_], FP32, kind="ExternalInput")
        refrac = nc.dram_tensor("refrac", [1, os_], FP32, kind="ExternalInput")
        # spikes are 0/1 -> exact in fp8; host casts back to fp32
        out = nc.dram_tensor("spikes", [os_, bs], FP8, kind="ExternalOutput")
        _TENSORS.update(
            spt=spt, synt=synt, thr=thr, vmem=vmem, refrac=refrac, out=out
        )
    else:
        spt, synt, thr, vmem, refrac, out = (
            _TENSORS["spt"],
            _TENSORS["synt"],
            _TENSORS["thr"],
            _TENSORS["vmem"],
            _TENSORS["refrac"],
            _TENSORS["out"],
        )

    NC_ = os_ // 128  # o-chunks (psum partition tiles)
    MP = in_ // 256  # m-pairs (K=256 per DoubleRow matmul)
    NBB = bs // 512  # moving-dim tiles
    SC = in_ // 512  # spike DMA chunks (4 m-chunks / 2 m-pairs each)
    WC = in_ // 512  # weight chunks (4 m-chunks / 2 m-pairs each)

    with (
        tc.tile_pool(name="spool", bufs=1) as spool,
        tc.tile_pool(name="wpool", bufs=1) as wpool,
        tc.tile_pool(name="synpool", bufs=3) as synpool,
        tc.tile_pool(name="tpool", bufs=3) as tpool,
        tc.tile_pool(name="outpool", bufs=4) as outpool,
        tc.tile_pool(name="miscpool", bufs=1) as miscpool,
        tc.tile_pool(name="pspool", bufs=8, space="PSUM") as pspool,
    ):
        # ---- negT[o] = -(effective threshold), computed in [8, 128] then
        # rearrange-DMA'd to per-partition layout negT_t[128, 8] ----
        tv = miscpool.tile([8, 128], FP32, tag="tv")
        vv = miscpool.tile([8, 128], FP32, tag="vv")
        rv = miscpool.tile([8, 128], FP32, tag="rv")
        nc.sync.dma_start(tv[:], thr[:, :].rearrange("a (c p) -> (a c) p", p=128))
        nc.sync.dma_start(vv[:], vmem[:, :].rearrange("a (c p) -> (a c) p", p=128))
        nc.sync.dma_start(rv[:], refrac[:, :].rearrange("a (c p) -> (a c) p", p=128))
        c0 = miscpool.tile([8, 128], FP32, tag="c0")
        nc.vector.tensor_scalar(c0[:], vv[:], DECAY, None, Op.mult)
        nc.vector.tensor_tensor(c0[:], c0[:], tv[:], Op.subtract)  # decay*v - thr
        big = miscpool.tile([8, 128], FP32, tag="big")
        nc.vector.tensor_scalar(big[:], c0[:], 0.0, None, Op.is_ge)
        nc.vector.tensor_scalar(big[:], big[:], 2e30, -1e30, Op.mult, Op.add)
        r01 = miscpool.tile([8, 128], FP32, tag="r01")
        nc.vector.tensor_scalar(r01[:], rv[:], 0.0, None, Op.is_gt)
        # negT = c0 + r01 * (big - c0)
        nc.vector.tensor_tensor(big[:], big[:], c0[:], Op.subtract)
        nc.vector.tensor_tensor(big[:], big[:], r01[:], Op.mult)
        negT = miscpool.tile([8, 128], FP32, tag="negT")
        nc.vector.tensor_tensor(negT[:], c0[:], big[:], Op.add)
        # per-partition layout negT_t[p, c] = negT[o = c*128 + p], via the
        # hardware 2-byte transpose unit (bf16: 0.5 thr exact, 1e30 in range)
        BF16 = mybir.dt.bfloat16
        ntb = miscpool.tile([32, 128], BF16, tag="ntb")
        nc.vector.memset(ntb[:], 0.0)
        nc.vector.tensor_scalar(ntb[0:8, :], negT[:], 0.0, None, Op.add)
        ntt = miscpool.tile([128, 32], BF16, tag="ntt")
        nc.sync.dma_start(ntt[:], ntb[:], transpose=True)
        negT_t = miscpool.tile([128, 8], FP32, tag="negT_t")
        nc.vector.tensor_scalar(negT_t[:], ntt[:, 0:8], 0.0, None, Op.add)

        # ---- interleaved loads: syn chunks feed the ternarize pipeline in
        # consumption order; spike chunks trickle in between on their own
        # queue. W' = negated ternary, 2 ops per chunk, split DVE/Pool ----
        Ss = [
            spool.tile([128, 4, bs], FP8, tag=f"S{c}", name=f"S{c}")
            for c in range(SC)
        ]
        Ws = [
            wpool.tile([128, 4, os_], FP8, tag=f"W{c}", name=f"W{c}")
            for c in range(WC)
        ]
        for c in range(WC):
            st = synpool.tile([128, 4, os_], FP32, tag="st", name="st")
            nc.sync.dma_start(
                st[:], synt[ts(c, 512), :].rearrange("(m p) o -> p m o", p=128)
            )
            nc.scalar.dma_start(
                Ss[c][:], spt[ts(c, 512), :].rearrange("(m p) b -> p m b", p=128)
            )
            # Pool does the compare (scalar_tensor_tensor is PE-illegal on
            # Pool), DVE the fused second compare + subtract
            ta = tpool.tile([128, 4, os_], FP8, tag="ta", name="ta")
            nc.gpsimd.tensor_scalar(ta[:], st[:], 1.0, None, Op.is_gt)
            # W' = (st < -1) - (st > 1)  == -ternarize(st)
            nc.vector.scalar_tensor_tensor(
                Ws[c][:], st[:], -1.0, ta[:], Op.is_lt, Op.subtract
            )

        # ---- main sweep: psum groups of 2 o-chunks x 4 moving tiles ----
        for og in range(NC_ // 2):
            pss = [
                [pspool.tile([128, 512], FP32, tag="ps", name="ps") for _ in range(NBB)]
                for _ in range(2)
            ]
            for m in range(2 * MP):
                wc, wj = divmod(m, 4)  # W chunk, m-chunk within chunk
                sc, sj = divmod(m, 4)  # S chunk, m-chunk within chunk
                for oi in range(2):
                    oc = og * 2 + oi
                    lhsT = Ws[wc][:, wj, ts(oc, 128)]
                    for bb in range(NBB):
                        nc.tensor.matmul(
                            pss[oi][bb][:],
                            lhsT,
                            Ss[sc][:, sj, ts(bb, 512)],
                            start=(m == 0),
                            stop=(m == 2 * MP - 1),
                        )
            for oi in range(2):
                oc = og * 2 + oi
                ob = outpool.tile([128, NBB, 512], FP8, tag="ob", name="ob")
                for bb in range(NBB):
                    # psum = -current; spike = (current >= T) == (psum <= -T)
                    nc.vector.tensor_scalar(
                        ob[:, bb, :],
                        pss[oi][bb][:],
                        negT_t[:, oc : oc + 1],
                        None,
                        Op.is_le,
                    )
                nc.scalar.dma_start(out[ts(oc, 128), :], ob[:])
    return out


def make_nc(bs=B // GB, os_=OUT // GO, in_=IN, repeat=1):
    from concourse import bacc
    from concourse.tile import TileContext

    nc = bacc.Bacc(trn_type="TRN2")
    with TileContext(nc) as tc:
        for r in range(repeat):
            build_core_program(nc, tc, bs, os_, in_, instance=r)
    nc.compile()
    return nc


def make_in_maps(
    spike_input,
    synapse_states,
    membrane_potential,
    adaptive_threshold,
    refractory_count,
):
    import ml_dtypes

    FP8 = ml_dtypes.float8_e4m3

    spike_input = np.asarray(spike_input, dtype=np.float32)
    synapse_states = np.asarray(synapse_states, dtype=np.float32)
    membrane_potential = np.asarray(membrane_potential, dtype=np.float32)
    adaptive_threshold = np.asarray(adaptive_threshold, dtype=np.float32)
    refractory_count = np.asarray(refractory_count, dtype=np.float32)

    bs, os_ = B // GB, OUT // GO
    # spike values are exactly {0.0, 1.0} -> fp8 cast is lossless
    spt_all = [
        np.ascontiguousarray(spike_input[bi * bs : (bi + 1) * bs].astype(FP8).T)
        for bi in range(GB)
    ]
    synt_all = [
        np.ascontiguousarray(synapse_states[oj * os_ : (oj + 1) * os_].T)
        for oj in range(GO)
    ]
    in_maps = []
    for c in range(GB * GO):
        bi, oj = divmod(c, GO)
        in_maps.append(
            {
                "spt": spt_all[bi],
                "synt": synt_all[oj],
                "thr": adaptive_threshold[None, oj * os_ : (oj + 1) * os_],
                "vmem": membrane_potential[None, oj * os_ : (oj + 1) * os_],
                "refrac": refractory_count[None, oj * os_ : (oj + 1) * os_],
            }
        )
    return in_maps


_NC_CACHE = {}


def kernel(
    spike_input,
    synapse_states,
    membrane_potential,
    adaptive_threshold,
    refractory_count,
    _return_results=False,
):
    from concourse.bass_utils import run_bass_kernel_spmd

    bs, os_ = B // GB, OUT // GO
    if "nc" not in _NC_CACHE:
        _NC_CACHE["nc"] = make_nc(bs, os_, IN)
    nc = _NC_CACHE["nc"]

    in_maps = make_in_maps(
        spike_input,
        synapse_states,
        membrane_potential,
        adaptive_threshold,
        refractory_count,
    )

    res = run_bass_kernel_spmd(nc, in_maps, core_ids=list(range(GB * GO)))

    full = np.empty((B, OUT), dtype=np.float32)
    for c in range(GB * GO):
        bi, oj = divmod(c, GO)
        # per-core output is [o, b]; transpose back
        full[bi * bs : (bi + 1) * bs, oj * os_ : (oj + 1) * os_] = (
            res.results[c]["spikes"].T.astype(np.float32)
        )
    if _return_results:
        return full, res
    return full
